# revision 1
# baseline (speedup 1.0000x reference)
"""LocalityEnhancedAttention Trainium2 kernel (8 NeuronCores, SPMD).

Sharding: core c handles batch b = c//2 and head-group g = c%2
(8 of 16 heads). Each core computes its partial output projection
(attn_heads @ wo_shard.T); host sums the two head-group partials per
batch and adds bo.

Device dataflow per core (S=2048, M=1024, local head-dims DH=512):
  - host pre-transposes inputs:  qT/kT/vT = x[b].T  [M, S]
  - projections (f32r matmuls): Q^T,K^T [DH, S] in [d, s] layout,
    V in [s, d] layout augmented with a ones column per head (rowsums)
  - scores^T[kj, qi] = K^T.T @ Q^T per head, head pairs packed into
    PE row-halves (contraction = Dk = 64), banded local bias added via
    DVE, exp via ACT (scale=1/8 folded in), P^T in SBUF
  - PV: A^T_aug[65, qi] += V_aug[kj].T @ P^T[kj] accumulated in PSUM;
    row 64 = softmax denominators.  Normalize via DVE recip +
    gpsimd partition_broadcast + DVE mul.
  - WO: out_partial[s, m] = sum_d A_norm^T.T @ woT
"""

import os
import sys
from contextlib import ExitStack

import numpy as np

sys.path.insert(0, "/opt/trn_rl_repo")

import ml_dtypes

BF = ml_dtypes.bfloat16

import concourse.bass as bass
import concourse.mybir as mybir
import concourse.tile as tile
from concourse import bacc
from concourse.bass_utils import run_bass_kernel_spmd

F32 = mybir.dt.float32
F32R = mybir.dt.float32r
BF16 = mybir.dt.bfloat16
EXP = mybir.ActivationFunctionType.Exp
TS, DS = bass.ts, bass.ds

S = 2048
M = 1024
DH = 512        # head dims per core (8 heads x 64)
DK = 64
W = 16
NPT = 4         # head pairs per core
NCH = 4         # qi chunks of 512
NKJ = 16        # kj tiles of 128


def _emit(ctx, tc, io):
    nc = tc.nc

    const = ctx.enter_context(tc.tile_pool(name="const", bufs=1))
    qkvp = ctx.enter_context(tc.tile_pool(name="qkv", bufs=1))
    ap_ = ctx.enter_context(tc.tile_pool(name="anorm", bufs=1))
    wop = ctx.enter_context(tc.tile_pool(name="wop", bufs=1))

    pat = const.tile([128, 320], BF16, tag="pat", name="pat")
    nc.sync.dma_start(pat[:], io["pat"])
    ones_r = const.tile([1, 512], BF16, tag="ones_r", name="ones_r")
    nc.vector.memset(ones_r[:], 1.0)
    bq = const.tile([1, DH], BF16, tag="bq", name="bq")
    nc.sync.dma_start(bq[:], io["bq"])
    bk = const.tile([1, DH], BF16, tag="bk", name="bk")
    nc.sync.dma_start(bk[:], io["bk"])
    bv = const.tile([1, DH], BF16, tag="bv", name="bv")
    nc.sync.dma_start(bv[:], io["bv"])

    qT_sb = [qkvp.tile([128, S], BF16, tag=f"q{i}", name=f"q{i}") for i in range(NPT)]
    kT_sb = [qkvp.tile([128, S], BF16, tag=f"k{i}", name=f"k{i}") for i in range(NPT)]
    v_sb = [qkvp.tile([128, 8 * 65], BF16, tag=f"v{i}", name=f"v{i}") for i in range(16)]
    a_sb = [ap_.tile([128, S], BF16, tag=f"a{i}", name=f"a{i}") for i in range(NPT)]
    woT_sb = [wop.tile([128, M], BF16, tag=f"wo{i}", name=f"wo{i}") for i in range(NPT)]
    for i in range(NPT):
        nc.sync.dma_start(woT_sb[i][:], io["woT"][TS(i, 128), :])

    # ---------------- projections ----------------
    with ExitStack() as ps:
        wpool = ps.enter_context(tc.tile_pool(name="wpool", bufs=10))
        stream = ps.enter_context(tc.tile_pool(name="stream", bufs=12))
        pproj = ps.enter_context(tc.tile_pool(name="pproj", bufs=2, space="PSUM"))

        # V projection -> v_sb[st] = [128, 8*65] ([s, head-major d | ones])
        wv = []
        for k in range(8):
            t = wpool.tile([128, DH], BF16, tag="w", name="w")
            nc.sync.dma_start(t[:], io["wvT"][TS(k, 128), :])
            wv.append(t)
        for st in range(16):
            vv = v_sb[st].rearrange("p (h e) -> p h e", e=65)
            nc.vector.memset(vv[:, :, 64:65], 1.0)
        for sc in range(4):  # s-chunks of 512
            vs = []
            for k in range(8):
                t = stream.tile([128, 512], BF16, tag="s", name="s")
                nc.sync.dma_start(t[:], io["vT"][TS(k, 128), TS(sc, 512)])
                vs.append(t)
            for j in range(4):
                st = sc * 4 + j
                psv = pproj.tile([128, DH], F32, tag="pp", name="pp")
                for k in range(8):
                    nc.tensor.matmul(
                        psv[:],
                        lhsT=vs[k][:, TS(j, 128)],
                        rhs=wv[k][:],
                        start=(k == 0), stop=False, skip_group_check=True)
                nc.tensor.matmul(
                    psv[:],
                    lhsT=ones_r[0:1, 0:128],
                    rhs=bv[:],
                    start=False, stop=True, skip_group_check=True)
                vv = v_sb[st].rearrange("p (h e) -> p h e", e=65)
                pv_view = psv.rearrange("p (h e) -> p h e", e=64)
                nc.vector.tensor_copy(vv[:, :, 0:64], pv_view[:, :, :])

        # Q^T and K^T projections -> [d, s] layout pair tiles
        for (w_name, x_name, b_tile, dst) in (
            ("wqT", "qT", bq, qT_sb),
            ("wkT", "kT", bk, kT_sb),
        ):
            wt = []
            for k in range(8):
                t = wpool.tile([128, DH], BF16, tag="w", name="w")
                nc.sync.dma_start(t[:], io[w_name][TS(k, 128), :])
                wt.append(t)
            for sc in range(4):
                xs = []
                for k in range(8):
                    t = stream.tile([128, 512], BF16, tag="s", name="s")
                    nc.sync.dma_start(t[:], io[x_name][TS(k, 128), TS(sc, 512)])
                    xs.append(t)
                for pt in range(NPT):
                    psq = pproj.tile([128, 512], F32, tag="pp", name="pp")
                    for k in range(8):
                        nc.tensor.matmul(
                            psq[:],
                            lhsT=wt[k][:, TS(pt, 128)],
                            rhs=xs[k][:],
                            start=(k == 0), stop=False, skip_group_check=True)
                    nc.tensor.matmul(
                        psq[:],
                        lhsT=b_tile[0:1, TS(pt, 128)],
                        rhs=ones_r[:],
                        start=False, stop=True, skip_group_check=True)
                    nc.vector.tensor_copy(dst[pt][:, TS(sc, 512)], psq[:])

    # ---------------- attention + output projection ----------------
    with ExitStack() as asx:
        ptp = asx.enter_context(tc.tile_pool(name="ptp", bufs=4))
        rp = asx.enter_context(tc.tile_pool(name="rp", bufs=6))
        op = asx.enter_context(tc.tile_pool(name="op", bufs=4))
        sps = asx.enter_context(tc.tile_pool(name="sps", bufs=2, space="PSUM"))
        pvs = asx.enter_context(tc.tile_pool(name="pvs", bufs=2, space="PSUM"))
        wops = asx.enter_context(tc.tile_pool(name="wops", bufs=2, space="PSUM"))

        patv = pat.rearrange("p (h w) -> p h w", h=2)
        for ch in range(NCH):
            c0 = ch * 512
            for pt in range(NPT):
                pvt = [pvs.tile([65, 512], F32, tag="pv", name="pv") for _ in range(2)]
                for kj in range(16):
                    kj0 = kj * 128
                    sp = sps.tile([128, 1024], F32, tag="sp", name="sp")
                    for h in (0, 1):
                        nc.tensor.matmul(
                            sp[:, DS(h * 512, 512)],
                            lhsT=kT_sb[pt][DS(h * 64, 64), TS(kj, 128)],
                            rhs=qT_sb[pt][DS(h * 64, 64), TS(ch, 512)],
                            start=True, stop=True,
                            tile_position=(h * 64, 0),
                            skip_group_check=True)
                    ptt = ptp.tile([128, 1024], BF16, tag="ptt", name="ptt")
                    nc.scalar.activation(ptt[:], sp[:], EXP, scale=0.125)
                    lo = max(kj0 - W, c0)
                    hi = min(kj0 + 128 + W, c0 + 512)
                    if lo < hi:
                        pa = lo - (kj0 - W)
                        pv3 = ptt.rearrange("p (h w) -> p h w", h=2)
                        nc.vector.tensor_mul(
                            pv3[:, :, DS(lo - c0, hi - lo)],
                            pv3[:, :, DS(lo - c0, hi - lo)],
                            patv[:, :, DS(pa, hi - lo)])
                    for h in (0, 1):
                        lh = pt * 2 + h
                        nc.tensor.matmul(
                            pvt[h][:],
                            lhsT=v_sb[kj][:, DS(lh * 65, 65)],
                            rhs=ptt[:, DS(h * 512, 512)],
                            start=(kj == 0), stop=(kj == 15),
                            skip_group_check=True)
                for h in (0, 1):
                    r1 = rp.tile([1, 512], F32, tag="r1", name="r1")
                    nc.vector.reciprocal(r1[:], pvt[h][DS(64, 1), :])
                    rb = rp.tile([64, 512], F32, tag="rb", name="rb")
                    nc.gpsimd.partition_broadcast(rb[:], r1[:])
                    nc.vector.tensor_mul(
                        a_sb[pt][DS(h * 64, 64), TS(ch, 512)],
                        pvt[h][DS(0, 64), :], rb[:])
            for j in range(4):
                st = ch * 4 + j
                for mt in range(2):
                    pso = wops.tile([128, 512], F32, tag="pso", name="pso")
                    for pt in range(NPT):
                        nc.tensor.matmul(
                            pso[:],
                            lhsT=a_sb[pt][:, TS(st, 128)],
                            rhs=woT_sb[pt][:, TS(mt, 512)],
                            start=(pt == 0), stop=(pt == 3),
                            skip_group_check=True)
                    ot = op.tile([128, 512], F32, tag="ot", name="ot")
                    nc.vector.tensor_copy(ot[:], pso[:])
                    nc.sync.dma_start(io["out"][TS(st, 128), TS(mt, 512)], ot[:])


_CACHE = {}


def _build():
    if "nc" in _CACHE:
        return _CACHE["nc"]
    nc = bacc.Bacc("TRN2", target_bir_lowering=False, debug=False)
    io = {}
    for name, shape in (
        ("qT", [M, S]), ("kT", [M, S]), ("vT", [M, S]),
        ("wqT", [M, DH]), ("wkT", [M, DH]), ("wvT", [M, DH]),
        ("woT", [DH, M]),
        ("bq", [1, DH]), ("bk", [1, DH]), ("bv", [1, DH]),    ):
        io[name] = nc.dram_tensor(name, shape, BF16, kind="ExternalInput").ap()
    io["pat"] = nc.dram_tensor("pat", [128, 320], BF16, kind="ExternalInput").ap()
    io["out"] = nc.dram_tensor("out", [S, M], F32, kind="ExternalOutput").ap()
    with tile.TileContext(nc) as tc:
        with ExitStack() as ctx:
            _emit(ctx, tc, io)
    nc.compile()
    _CACHE["nc"] = nc
    return nc


def _bias_pattern(local_bias):
    # multiplicative band pattern: exp(2*b[qi-kj+W]) inside the band, 1.0
    # outside; duplicated side by side for the two heads of a pair tile.
    p = np.arange(128)[:, None]
    f = np.arange(160)[None, :]
    idx = f - p  # rel + W
    valid = (idx >= 0) & (idx <= 2 * W)
    b = np.asarray(local_bias, np.float64)
    pat = np.where(valid, np.exp(2.0 * b[np.clip(idx, 0, 2 * W)]), 1.0)
    pat2 = np.concatenate([pat, pat], axis=1)
    return np.ascontiguousarray(pat2).astype(BF)


def kernel(query, key, value, wq, bq, wk, bk, wv, bv, wo, bo, local_bias):
    query = np.asarray(query, np.float32)
    key = np.asarray(key, np.float32)
    value = np.asarray(value, np.float32)
    wq, wk, wv, wo = (np.asarray(x, np.float32) for x in (wq, wk, wv, wo))
    bq, bk, bv, bo = (np.asarray(x, np.float32) for x in (bq, bk, bv, bo))
    pat = _bias_pattern(local_bias)

    nc = _build()
    in_maps = []
    for c in range(8):
        b, g = c // 2, c % 2
        sl = slice(g * DH, (g + 1) * DH)
        in_maps.append({
            "qT": np.ascontiguousarray(query[b].T).astype(BF),
            "kT": np.ascontiguousarray(key[b].T).astype(BF),
            "vT": np.ascontiguousarray(value[b].T).astype(BF),
            "wqT": np.ascontiguousarray(wq[sl, :].T).astype(BF),
            "wkT": np.ascontiguousarray(wk[sl, :].T).astype(BF),
            "wvT": np.ascontiguousarray(wv[sl, :].T).astype(BF),
            "woT": np.ascontiguousarray(wo[:, sl].T).astype(BF),
            "bq": np.ascontiguousarray(bq[sl]).reshape(1, DH).astype(BF),
            "bk": np.ascontiguousarray(bk[sl]).reshape(1, DH).astype(BF),
            "bv": np.ascontiguousarray(bv[sl]).reshape(1, DH).astype(BF),
            "pat": pat,
        })
    res = run_bass_kernel_spmd(
        nc, in_maps, core_ids=list(range(8)),
        trace=bool(int(os.environ.get("KERNEL_TRACE", "0"))),
    )
    _CACHE["last_result"] = res
    outs = [r["out"] for r in res.results]
    out = np.stack([outs[2 * b] + outs[2 * b + 1] + bo for b in range(4)])
    return out.astype(np.float32)



# revision 7
# speedup vs baseline: 1.1578x; 1.1578x over previous
"""LocalityEnhancedAttention Trainium2 kernel (8 NeuronCores, SPMD).

Sharding: core c handles batch b = c//2 and head-group g = c%2
(8 of 16 heads). Each core computes its partial output projection
(attn_heads @ wo_shard.T); host sums the two head-group partials per
batch and adds bo.

Device dataflow per core (S=2048, M=1024, local head-dims DH=512):
  - host pre-transposes inputs:  qT/kT/vT = x[b].T  [M, S]
  - projections: Q^T,K^T [DH, S] in [d, s] layout, V in [s, d] layout
    augmented with a ones column per head (rowsums for softmax denom)
  - scores^T[kj, qi] = K^T.T @ Q^T per head, head pairs packed into
    PE row-halves (contraction = Dk = 64), banded local bias applied
    multiplicatively post-exp via DVE, exp via ACT (scale=1/8 folded)
  - PV: A^T_aug[65, qi] += V_aug[kj].T @ P^T[kj] accumulated in PSUM;
    row 64 = softmax denominators.
  - normalization (batched per qi-chunk): denom rows gathered to one
    [8, 512] tile, single DVE reciprocal, PE selector-matmul broadcast
    to [128, 512], in-place DVE muls on a_sb.
  - WO: out_partial[s, m] = sum_d A_norm^T.T @ woT

Emission is software-pipelined: K proj -> Q proj(ch0) -> group(0,0)
scores/exp -> V proj (overlaps exps) -> PV burst -> steady groups with
PV lagging QK by 2 tiles; Q proj(ch+1) / norm(ch) / WO(ch-1) are
stitched between attention groups so ACT (the bottleneck engine at
~270us of exp work) never starves.
"""

import os
import sys
from contextlib import ExitStack

import numpy as np

sys.path.insert(0, "/opt/trn_rl_repo")

import ml_dtypes

BF = ml_dtypes.bfloat16

import concourse.bass as bass
import concourse.mybir as mybir
import concourse.tile as tile
from concourse import bacc
from concourse.bass_utils import run_bass_kernel_spmd

F32 = mybir.dt.float32
BF16 = mybir.dt.bfloat16
EXP = mybir.ActivationFunctionType.Exp
TS, DS = bass.ts, bass.ds

S = 2048
M = 1024
DH = 512        # head dims per core (8 heads x 64)
DK = 64
W = 16
NPT = 4         # head pairs per core
NCH = 4         # qi chunks of 512
NKJ = 16        # kj tiles of 128
LAG = 2         # PV trails QK/exp by this many kj tiles


def _emit(ctx, tc, io):
    nc = tc.nc

    const = ctx.enter_context(tc.tile_pool(name="const", bufs=1))
    qkvp = ctx.enter_context(tc.tile_pool(name="qkv", bufs=1))
    stream = ctx.enter_context(tc.tile_pool(name="stream", bufs=16))
    ptp = ctx.enter_context(tc.tile_pool(name="ptp", bufs=20))
    normp = ctx.enter_context(tc.tile_pool(name="normp", bufs=2))
    op = ctx.enter_context(tc.tile_pool(name="op", bufs=4))
    sps = ctx.enter_context(tc.tile_pool(name="sps", bufs=2, space="PSUM"))
    pvs = ctx.enter_context(tc.tile_pool(name="pvs", bufs=2, space="PSUM"))
    wops = ctx.enter_context(tc.tile_pool(name="wops", bufs=2, space="PSUM"))

    # ---------------- constants ----------------
    pat = const.tile([128, 320], BF16, tag="pat", name="pat")
    nc.sync.dma_start(pat[:], io["pat"])
    ones_r = const.tile([1, 512], BF16, tag="ones_r", name="ones_r")
    nc.vector.memset(ones_r[:], 1.0)
    bq = const.tile([1, DH], BF16, tag="bq", name="bq")
    nc.sync.dma_start(bq[:], io["bq"])
    bk = const.tile([1, DH], BF16, tag="bk", name="bk")
    nc.sync.dma_start(bk[:], io["bk"])
    bv = const.tile([1, DH], BF16, tag="bv", name="bv")
    nc.sync.dma_start(bv[:], io["bv"])

    woT_sb = [const.tile([128, M], BF16, tag=f"wo{i}", name=f"wo{i}") for i in range(NPT)]
    for i in range(NPT):
        nc.sync.dma_start(woT_sb[i][:], io["woT"][TS(i, 128), :])
    wq_sb = [const.tile([128, DH], BF16, tag=f"wq{k}", name=f"wq{k}") for k in range(8)]
    wk_sb = [const.tile([128, DH], BF16, tag=f"wk{k}", name=f"wk{k}") for k in range(8)]
    wv_sb = [const.tile([128, DH], BF16, tag=f"wv{k}", name=f"wv{k}") for k in range(8)]
    for k in range(8):
        nc.sync.dma_start(wk_sb[k][:], io["wkT"][TS(k, 128), :])
        nc.sync.dma_start(wq_sb[k][:], io["wqT"][TS(k, 128), :])
        nc.sync.dma_start(wv_sb[k][:], io["wvT"][TS(k, 128), :])

    # ---------------- persistent tiles ----------------
    qT_sb = [qkvp.tile([128, S], BF16, tag=f"q{i}", name=f"q{i}") for i in range(NPT)]
    kT_sb = [qkvp.tile([128, S], BF16, tag=f"k{i}", name=f"k{i}") for i in range(NPT)]
    v_sb = [qkvp.tile([128, 8 * 65], BF16, tag=f"v{i}", name=f"v{i}") for i in range(16)]
    a_sb = [qkvp.tile([128, S], BF16, tag=f"a{i}", name=f"a{i}") for i in range(NPT)]
    for st in range(16):
        vv = v_sb[st].rearrange("p (h e) -> p h e", e=65)
        nc.vector.memset(vv[:, :, 64:65], 1.0)

    patv = pat.rearrange("p (h w) -> p h w", h=2)

    # ---------------- projection emitters ----------------
    def proj_qk(w_sb, x_name, b_tile, dst, sc):
        xs = []
        for k in range(8):
            t = stream.tile([128, 512], BF16, tag="s", name="s")
            nc.sync.dma_start(t[:], io[x_name][TS(k, 128), TS(sc, 512)])
            xs.append(t)
        for pt in range(NPT):
            ps = wops.tile([128, 512], F32, tag="ps", name="ps")
            for k in range(8):
                nc.tensor.matmul(
                    ps[:], lhsT=w_sb[k][:, TS(pt, 128)], rhs=xs[k][:],
                    start=(k == 0), stop=False, skip_group_check=True)
            nc.tensor.matmul(
                ps[:], lhsT=b_tile[0:1, TS(pt, 128)], rhs=ones_r[:],
                start=False, stop=True, skip_group_check=True)
            nc.vector.tensor_copy(dst[pt][:, TS(sc, 512)], ps[:])

    def proj_v(sc):
        xs = []
        for k in range(8):
            t = stream.tile([128, 512], BF16, tag="s", name="s")
            nc.sync.dma_start(t[:], io["vT"][TS(k, 128), TS(sc, 512)])
            xs.append(t)
        for j in range(4):
            st = sc * 4 + j
            ps = wops.tile([128, DH], F32, tag="ps", name="ps")
            for k in range(8):
                nc.tensor.matmul(
                    ps[:], lhsT=xs[k][:, TS(j, 128)], rhs=wv_sb[k][:],
                    start=(k == 0), stop=False, skip_group_check=True)
            nc.tensor.matmul(
                ps[:], lhsT=ones_r[0:1, 0:128], rhs=bv[:],
                start=False, stop=True, skip_group_check=True)
            vv = v_sb[st].rearrange("p (h e) -> p h e", e=65)
            pv_view = ps.rearrange("p (h e) -> p h e", e=64)
            nc.vector.tensor_copy(vv[:, :, 0:64], pv_view[:, :, :])

    # ---------------- attention emitters ----------------
    def qk_exp(ch, pt, kj):
        c0, kj0 = ch * 512, kj * 128
        sp = sps.tile([128, 1024], F32, tag="sp", name="sp")
        for h in (0, 1):
            nc.tensor.matmul(
                sp[:, DS(h * 512, 512)],
                lhsT=kT_sb[pt][DS(h * 64, 64), TS(kj, 128)],
                rhs=qT_sb[pt][DS(h * 64, 64), TS(ch, 512)],
                start=True, stop=True,
                tile_position=(h * 64, 0),
                skip_group_check=True)
        ptt = ptp.tile([128, 1024], BF16, tag="ptt", name="ptt")
        nc.scalar.activation(ptt[:], sp[:], EXP, scale=0.125)
        lo = max(kj0 - W, c0)
        hi = min(kj0 + 128 + W, c0 + 512)
        if lo < hi:
            pa = lo - (kj0 - W)
            pv3 = ptt.rearrange("p (h w) -> p h w", h=2)
            nc.vector.tensor_mul(
                pv3[:, :, DS(lo - c0, hi - lo)],
                pv3[:, :, DS(lo - c0, hi - lo)],
                patv[:, :, DS(pa, hi - lo)])
        return ptt

    def pv(pt, kj, ptt, pvt):
        for h in (0, 1):
            nc.tensor.matmul(
                pvt[h][:],
                lhsT=v_sb[kj][:, DS((pt * 2 + h) * 65, 65)],
                rhs=ptt[:, DS(h * 512, 512)],
                start=(kj == 0), stop=(kj == 15),
                skip_group_check=True)

    def finish_group(ch, pt, pvt):
        # reciprocal of the denominator row (PSUM partition 64), then
        # broadcast to 64 partitions on the idle GpSimd engine.  The
        # a_sb write-out muls are deferred into the next group so the
        # DVE queue never blocks on the gpsimd broadcast.
        rbs = []
        for h in (0, 1):
            rf = normp.tile([1, 512], F32, tag="rf", name="rf", bufs=4)
            nc.vector.reciprocal(rf[:], pvt[h][DS(64, 1), :])
            rb = normp.tile([64, 512], F32, tag="rb", name="rb", bufs=4)
            nc.gpsimd.partition_broadcast(rb[:], rf[:])
            rbs.append(rb)
        return (ch, pt, pvt, rbs)

    def norm_muls(pending):
        if pending is None:
            return
        ch, pt, pvt, rbs = pending
        for h in (0, 1):
            nc.vector.tensor_mul(
                a_sb[pt][DS(h * 64, 64), TS(ch, 512)],
                pvt[h][DS(0, 64), :], rbs[h][:])

    def group(ch, pt, pending):
        pvt = [pvs.tile([65, 512], F32, tag="pv", name="pv") for _ in (0, 1)]
        live = {}
        for kj in range(NKJ + LAG):
            if kj < NKJ:
                live[kj] = qk_exp(ch, pt, kj)
            if kj == 1:
                norm_muls(pending)
            if kj >= LAG:
                pv(pt, kj - LAG, live.pop(kj - LAG), pvt)
        return finish_group(ch, pt, pvt)

    def wo(ch):
        for j in range(4):
            st = ch * 4 + j
            for mt in range(2):
                pso = wops.tile([128, 512], F32, tag="ps", name="ps")
                for pt in range(NPT):
                    nc.tensor.matmul(
                        pso[:],
                        lhsT=a_sb[pt][:, TS(st, 128)],
                        rhs=woT_sb[pt][:, TS(mt, 512)],
                        start=(pt == 0), stop=(pt == 3),
                        skip_group_check=True)
                ot = op.tile([128, 512], F32, tag="ot", name="ot")
                nc.vector.tensor_copy(ot[:], pso[:])
                nc.sync.dma_start(io["out"][TS(st, 128), TS(mt, 512)], ot[:])

    # ---------------- main schedule ----------------
    for sc in range(4):
        proj_qk(wk_sb, "kT", bk, kT_sb, sc)
    proj_qk(wq_sb, "qT", bq, qT_sb, 0)

    # group (0,0): all QK/exp first (feeds ACT), V projection overlaps
    # the exps on the Tensor engine, then the PV burst.
    live0 = [qk_exp(0, 0, kj) for kj in range(NKJ)]
    for sc in range(4):
        proj_v(sc)
    pvt0 = [pvs.tile([65, 512], F32, tag="pv", name="pv") for _ in (0, 1)]
    for kj in range(NKJ):
        pv(0, kj, live0[kj], pvt0)
    pending = finish_group(0, 0, pvt0)

    for g in range(1, 16):
        ch, pt = divmod(g, 4)
        pending = group(ch, pt, pending)
        if pt == 1 and ch < 3:
            proj_qk(wq_sb, "qT", bq, qT_sb, ch + 1)
        if pt == 2 and ch >= 1:
            wo(ch - 1)
    norm_muls(pending)
    wo(3)


_CACHE = {}


def _build():
    if "nc" in _CACHE:
        return _CACHE["nc"]
    nc = bacc.Bacc("TRN2", target_bir_lowering=False, debug=False)
    io = {}
    for name, shape in (
        ("qT", [M, S]), ("kT", [M, S]), ("vT", [M, S]),
        ("wqT", [M, DH]), ("wkT", [M, DH]), ("wvT", [M, DH]),
        ("woT", [DH, M]),
        ("bq", [1, DH]), ("bk", [1, DH]), ("bv", [1, DH]),    ):
        io[name] = nc.dram_tensor(name, shape, BF16, kind="ExternalInput").ap()
    io["pat"] = nc.dram_tensor("pat", [128, 320], BF16, kind="ExternalInput").ap()
    io["out"] = nc.dram_tensor("out", [S, M], F32, kind="ExternalOutput").ap()
    with tile.TileContext(nc) as tc:
        with ExitStack() as ctx:
            _emit(ctx, tc, io)
    nc.compile()
    _CACHE["nc"] = nc
    return nc


def _bias_pattern(local_bias):
    # multiplicative band pattern: exp(2*b[qi-kj+W]) inside the band, 1.0
    # outside; duplicated side by side for the two heads of a pair tile.
    p = np.arange(128)[:, None]
    f = np.arange(160)[None, :]
    idx = f - p  # rel + W
    valid = (idx >= 0) & (idx <= 2 * W)
    b = np.asarray(local_bias, np.float64)
    pat = np.where(valid, np.exp(2.0 * b[np.clip(idx, 0, 2 * W)]), 1.0)
    pat2 = np.concatenate([pat, pat], axis=1)
    return np.ascontiguousarray(pat2).astype(BF)


def kernel(query, key, value, wq, bq, wk, bk, wv, bv, wo, bo, local_bias):
    query = np.asarray(query, np.float32)
    key = np.asarray(key, np.float32)
    value = np.asarray(value, np.float32)
    wq, wk, wv, wo = (np.asarray(x, np.float32) for x in (wq, wk, wv, wo))
    bq, bk, bv, bo = (np.asarray(x, np.float32) for x in (bq, bk, bv, bo))
    pat = _bias_pattern(local_bias)

    nc = _build()
    in_maps = []
    for c in range(8):
        b, g = c // 2, c % 2
        sl = slice(g * DH, (g + 1) * DH)
        in_maps.append({
            "qT": np.ascontiguousarray(query[b].T).astype(BF),
            "kT": np.ascontiguousarray(key[b].T).astype(BF),
            "vT": np.ascontiguousarray(value[b].T).astype(BF),
            "wqT": np.ascontiguousarray(wq[sl, :].T).astype(BF),
            "wkT": np.ascontiguousarray(wk[sl, :].T).astype(BF),
            "wvT": np.ascontiguousarray(wv[sl, :].T).astype(BF),
            "woT": np.ascontiguousarray(wo[:, sl].T).astype(BF),
            "bq": np.ascontiguousarray(bq[sl]).reshape(1, DH).astype(BF),
            "bk": np.ascontiguousarray(bk[sl]).reshape(1, DH).astype(BF),
            "bv": np.ascontiguousarray(bv[sl]).reshape(1, DH).astype(BF),
            "pat": pat,
        })
    res = run_bass_kernel_spmd(
        nc, in_maps, core_ids=list(range(8)),
        trace=bool(int(os.environ.get("KERNEL_TRACE", "0"))),
    )
    _CACHE["last_result"] = res
    outs = [r["out"] for r in res.results]
    out = np.stack([outs[2 * b] + outs[2 * b + 1] + bo for b in range(4)])
    return out.astype(np.float32)


# revision 8
# speedup vs baseline: 1.4300x; 1.2351x over previous
"""LocalityEnhancedAttention Trainium2 kernel (8 NeuronCores, SPMD).

Sharding: core c handles batch b = c//2 and head-group g = c%2
(8 of 16 heads). Each core computes its partial output projection
(attn_heads @ wo_shard.T); host sums the two head-group partials per
batch and adds bo.

Device dataflow per core (S=2048, M=1024, local head-dims DH=512):
  - host pre-transposes inputs:  qT/kT/vT = x[b].T  [M, S]
  - projections: Q^T,K^T [DH, S] in [d, s] layout, V in [s, d] layout
    augmented with a ones column per head (rowsums for softmax denom)
  - scores^T[kj, qi] = K^T.T @ Q^T per head, head pairs packed into
    PE row-halves (contraction = Dk = 64), banded local bias applied
    multiplicatively post-exp via DVE, exp via ACT (scale=1/8 folded)
  - PV: A^T_aug[65, qi] += V_aug[kj].T @ P^T[kj] accumulated in PSUM;
    row 64 = softmax denominators.
  - normalization (batched per qi-chunk): denom rows gathered to one
    [8, 512] tile, single DVE reciprocal, PE selector-matmul broadcast
    to [128, 512], in-place DVE muls on a_sb.
  - WO: out_partial[s, m] = sum_d A_norm^T.T @ woT

Emission is software-pipelined: K proj -> Q proj(ch0) -> group(0,0)
scores/exp -> V proj (overlaps exps) -> PV burst -> steady groups with
PV lagging QK by 2 tiles; Q proj(ch+1) / norm(ch) / WO(ch-1) are
stitched between attention groups so ACT (the bottleneck engine at
~270us of exp work) never starves.
"""

import os
import sys
from contextlib import ExitStack

import numpy as np

sys.path.insert(0, "/opt/trn_rl_repo")

import ml_dtypes

BF = ml_dtypes.bfloat16

import concourse.bass as bass
import concourse.mybir as mybir
import concourse.tile as tile
from concourse import bacc
from concourse.bass_utils import run_bass_kernel_spmd

F32 = mybir.dt.float32
BF16 = mybir.dt.bfloat16
EXP = mybir.ActivationFunctionType.Exp
TS, DS = bass.ts, bass.ds

S = 2048
M = 1024
DH = 512        # head dims per core (8 heads x 64)
DK = 64
W = 16
NPT = 4         # head pairs per core
NCH = 4         # qi chunks of 512
NKJ = 16        # kj tiles of 128
LAG = 2         # PV trails QK/exp by this many kj tiles


def _emit(ctx, tc, io):
    nc = tc.nc

    const = ctx.enter_context(tc.tile_pool(name="const", bufs=1))
    qkvp = ctx.enter_context(tc.tile_pool(name="qkv", bufs=1))
    stream = ctx.enter_context(tc.tile_pool(name="stream", bufs=16))
    ptp = ctx.enter_context(tc.tile_pool(name="ptp", bufs=20))
    normp = ctx.enter_context(tc.tile_pool(name="normp", bufs=2))
    op = ctx.enter_context(tc.tile_pool(name="op", bufs=4))
    sps = ctx.enter_context(tc.tile_pool(name="sps", bufs=2, space="PSUM"))
    pvs = ctx.enter_context(tc.tile_pool(name="pvs", bufs=2, space="PSUM"))
    wops = ctx.enter_context(tc.tile_pool(name="wops", bufs=2, space="PSUM"))

    # ---------------- constants ----------------
    pat = const.tile([128, 320], BF16, tag="pat", name="pat")
    nc.sync.dma_start(pat[:], io["pat"])
    ones_r = const.tile([1, 512], BF16, tag="ones_r", name="ones_r")
    nc.vector.memset(ones_r[:], 1.0)
    bq = const.tile([1, DH], BF16, tag="bq", name="bq")
    nc.sync.dma_start(bq[:], io["bq"])
    bk = const.tile([1, DH], BF16, tag="bk", name="bk")
    nc.sync.dma_start(bk[:], io["bk"])
    bv = const.tile([1, DH], BF16, tag="bv", name="bv")
    nc.sync.dma_start(bv[:], io["bv"])

    woT_sb = [const.tile([128, M], BF16, tag=f"wo{i}", name=f"wo{i}") for i in range(NPT)]
    for i in range(NPT):
        nc.sync.dma_start(woT_sb[i][:], io["woT"][TS(i, 128), :])
    wq_sb = [const.tile([128, DH], BF16, tag=f"wq{k}", name=f"wq{k}") for k in range(8)]
    wk_sb = [const.tile([128, DH], BF16, tag=f"wk{k}", name=f"wk{k}") for k in range(8)]
    wv_sb = [const.tile([128, DH], BF16, tag=f"wv{k}", name=f"wv{k}") for k in range(8)]
    for k in range(8):
        nc.sync.dma_start(wk_sb[k][:], io["wkT"][TS(k, 128), :])
        nc.sync.dma_start(wq_sb[k][:], io["wqT"][TS(k, 128), :])
        nc.sync.dma_start(wv_sb[k][:], io["wvT"][TS(k, 128), :])

    # ---------------- persistent tiles ----------------
    qT_sb = [qkvp.tile([128, S], BF16, tag=f"q{i}", name=f"q{i}") for i in range(NPT)]
    kT_sb = [qkvp.tile([128, S], BF16, tag=f"k{i}", name=f"k{i}") for i in range(NPT)]
    v_sb = [qkvp.tile([128, 8 * 65], BF16, tag=f"v{i}", name=f"v{i}") for i in range(16)]
    a_sb = [qkvp.tile([128, S], BF16, tag=f"a{i}", name=f"a{i}") for i in range(NPT)]
    for st in range(16):
        vv = v_sb[st].rearrange("p (h e) -> p h e", e=65)
        nc.vector.memset(vv[:, :, 64:65], 1.0)

    patv = pat.rearrange("p (h w) -> p h w", h=2)

    # ---------------- projection emitters ----------------
    def proj_qk(w_sb, x_name, b_tile, dst, sc):
        xs = []
        for k in range(8):
            t = stream.tile([128, 512], BF16, tag="s", name="s")
            nc.sync.dma_start(t[:], io[x_name][TS(k, 128), TS(sc, 512)])
            xs.append(t)
        for pt in range(NPT):
            ps = wops.tile([128, 512], F32, tag="ps", name="ps")
            for k in range(8):
                nc.tensor.matmul(
                    ps[:], lhsT=w_sb[k][:, TS(pt, 128)], rhs=xs[k][:],
                    start=(k == 0), stop=False, skip_group_check=True)
            nc.tensor.matmul(
                ps[:], lhsT=b_tile[0:1, TS(pt, 128)], rhs=ones_r[:],
                start=False, stop=True, skip_group_check=True)
            nc.vector.tensor_copy(dst[pt][:, TS(sc, 512)], ps[:])

    def proj_v(sc):
        xs = []
        for k in range(8):
            t = stream.tile([128, 512], BF16, tag="s", name="s")
            nc.sync.dma_start(t[:], io["vT"][TS(k, 128), TS(sc, 512)])
            xs.append(t)
        for j in range(4):
            st = sc * 4 + j
            ps = wops.tile([128, DH], F32, tag="ps", name="ps")
            for k in range(8):
                nc.tensor.matmul(
                    ps[:], lhsT=xs[k][:, TS(j, 128)], rhs=wv_sb[k][:],
                    start=(k == 0), stop=False, skip_group_check=True)
            nc.tensor.matmul(
                ps[:], lhsT=ones_r[0:1, 0:128], rhs=bv[:],
                start=False, stop=True, skip_group_check=True)
            vv = v_sb[st].rearrange("p (h e) -> p h e", e=65)
            pv_view = ps.rearrange("p (h e) -> p h e", e=64)
            nc.vector.tensor_copy(vv[:, :, 0:64], pv_view[:, :, :])

    # ---------------- attention emitters ----------------
    def qk_exp(ch, pt, kj):
        c0, kj0 = ch * 512, kj * 128
        sp = sps.tile([128, 1024], F32, tag="sp", name="sp")
        for h in (0, 1):
            nc.tensor.matmul(
                sp[:, DS(h * 512, 512)],
                lhsT=kT_sb[pt][DS(h * 64, 64), TS(kj, 128)],
                rhs=qT_sb[pt][DS(h * 64, 64), TS(ch, 512)],
                start=True, stop=True,
                tile_position=(h * 64, 0),
                skip_group_check=True)
        ptt = ptp.tile([128, 1024], BF16, tag="ptt", name="ptt")
        nc.scalar.activation(ptt[:], sp[:], EXP, scale=0.125)
        lo = max(kj0 - W, c0)
        hi = min(kj0 + 128 + W, c0 + 512)
        if lo < hi:
            pa = lo - (kj0 - W)
            pv3 = ptt.rearrange("p (h w) -> p h w", h=2)
            nc.vector.tensor_mul(
                pv3[:, :, DS(lo - c0, hi - lo)],
                pv3[:, :, DS(lo - c0, hi - lo)],
                patv[:, :, DS(pa, hi - lo)])
        return ptt

    def pv(pt, kj, ptt, pvt):
        for h in (0, 1):
            nc.tensor.matmul(
                pvt[h][:],
                lhsT=v_sb[kj][:, DS((pt * 2 + h) * 65, 65)],
                rhs=ptt[:, DS(h * 512, 512)],
                start=(kj == 0), stop=(kj == 15),
                skip_group_check=True)

    def finish_group(ch, pt, pvt):
        # reciprocal of the denominator row (PSUM partition 64), then
        # broadcast to 64 partitions on the idle GpSimd engine.  The
        # a_sb write-out muls are deferred into the next group so the
        # DVE queue never blocks on the gpsimd broadcast.
        rbs = []
        for h in (0, 1):
            dt = normp.tile([1, 512], F32, tag="dt", name="dt", bufs=4)
            nc.vector.tensor_copy(dt[:], pvt[h][DS(64, 1), :])
            rf = normp.tile([1, 512], F32, tag="rf", name="rf", bufs=4)
            nc.vector.reciprocal_approx_fast(rf[:], dt[:])
            rb = normp.tile([64, 512], F32, tag="rb", name="rb", bufs=4)
            nc.gpsimd.partition_broadcast(rb[:], rf[:])
            rbs.append(rb)
        return (ch, pt, pvt, rbs)

    def norm_muls(pending):
        if pending is None:
            return
        ch, pt, pvt, rbs = pending
        for h in (0, 1):
            nc.vector.tensor_mul(
                a_sb[pt][DS(h * 64, 64), TS(ch, 512)],
                pvt[h][DS(0, 64), :], rbs[h][:])

    def group(ch, pt, pending):
        pvt = [pvs.tile([65, 512], F32, tag="pv", name="pv") for _ in (0, 1)]
        live = {}
        for kj in range(NKJ + LAG):
            if kj < NKJ:
                live[kj] = qk_exp(ch, pt, kj)
            if kj == 1:
                norm_muls(pending)
            if kj >= LAG:
                pv(pt, kj - LAG, live.pop(kj - LAG), pvt)
        return finish_group(ch, pt, pvt)

    def wo(ch):
        for j in range(4):
            st = ch * 4 + j
            for mt in range(2):
                pso = wops.tile([128, 512], F32, tag="ps", name="ps")
                for pt in range(NPT):
                    nc.tensor.matmul(
                        pso[:],
                        lhsT=a_sb[pt][:, TS(st, 128)],
                        rhs=woT_sb[pt][:, TS(mt, 512)],
                        start=(pt == 0), stop=(pt == 3),
                        skip_group_check=True)
                ot = op.tile([128, 512], F32, tag="ot", name="ot")
                nc.vector.tensor_copy(ot[:], pso[:])
                nc.sync.dma_start(io["out"][TS(st, 128), TS(mt, 512)], ot[:])

    # ---------------- main schedule ----------------
    for sc in range(4):
        proj_qk(wk_sb, "kT", bk, kT_sb, sc)
    proj_qk(wq_sb, "qT", bq, qT_sb, 0)

    # group (0,0): all QK/exp first (feeds ACT), V projection overlaps
    # the exps on the Tensor engine, then the PV burst.
    live0 = [qk_exp(0, 0, kj) for kj in range(NKJ)]
    for sc in range(4):
        proj_v(sc)
    pvt0 = [pvs.tile([65, 512], F32, tag="pv", name="pv") for _ in (0, 1)]
    for kj in range(NKJ):
        pv(0, kj, live0[kj], pvt0)
    pending = finish_group(0, 0, pvt0)

    for g in range(1, 16):
        ch, pt = divmod(g, 4)
        pending = group(ch, pt, pending)
        if pt == 1 and ch < 3:
            proj_qk(wq_sb, "qT", bq, qT_sb, ch + 1)
        if pt == 2 and ch >= 1:
            wo(ch - 1)
    norm_muls(pending)
    wo(3)


_CACHE = {}


def _build():
    if "nc" in _CACHE:
        return _CACHE["nc"]
    nc = bacc.Bacc("TRN2", target_bir_lowering=False, debug=False)
    io = {}
    for name, shape in (
        ("qT", [M, S]), ("kT", [M, S]), ("vT", [M, S]),
        ("wqT", [M, DH]), ("wkT", [M, DH]), ("wvT", [M, DH]),
        ("woT", [DH, M]),
        ("bq", [1, DH]), ("bk", [1, DH]), ("bv", [1, DH]),    ):
        io[name] = nc.dram_tensor(name, shape, BF16, kind="ExternalInput").ap()
    io["pat"] = nc.dram_tensor("pat", [128, 320], BF16, kind="ExternalInput").ap()
    io["out"] = nc.dram_tensor("out", [S, M], F32, kind="ExternalOutput").ap()
    with tile.TileContext(nc) as tc:
        with ExitStack() as ctx:
            _emit(ctx, tc, io)
    nc.compile()
    _CACHE["nc"] = nc
    return nc


def _bias_pattern(local_bias):
    # multiplicative band pattern: exp(2*b[qi-kj+W]) inside the band, 1.0
    # outside; duplicated side by side for the two heads of a pair tile.
    p = np.arange(128)[:, None]
    f = np.arange(160)[None, :]
    idx = f - p  # rel + W
    valid = (idx >= 0) & (idx <= 2 * W)
    b = np.asarray(local_bias, np.float64)
    pat = np.where(valid, np.exp(2.0 * b[np.clip(idx, 0, 2 * W)]), 1.0)
    pat2 = np.concatenate([pat, pat], axis=1)
    return np.ascontiguousarray(pat2).astype(BF)


def kernel(query, key, value, wq, bq, wk, bk, wv, bv, wo, bo, local_bias):
    query = np.asarray(query, np.float32)
    key = np.asarray(key, np.float32)
    value = np.asarray(value, np.float32)
    wq, wk, wv, wo = (np.asarray(x, np.float32) for x in (wq, wk, wv, wo))
    bq, bk, bv, bo = (np.asarray(x, np.float32) for x in (bq, bk, bv, bo))
    pat = _bias_pattern(local_bias)

    nc = _build()
    in_maps = []
    for c in range(8):
        b, g = c // 2, c % 2
        sl = slice(g * DH, (g + 1) * DH)
        in_maps.append({
            "qT": np.ascontiguousarray(query[b].T).astype(BF),
            "kT": np.ascontiguousarray(key[b].T).astype(BF),
            "vT": np.ascontiguousarray(value[b].T).astype(BF),
            "wqT": np.ascontiguousarray(wq[sl, :].T).astype(BF),
            "wkT": np.ascontiguousarray(wk[sl, :].T).astype(BF),
            "wvT": np.ascontiguousarray(wv[sl, :].T).astype(BF),
            "woT": np.ascontiguousarray(wo[:, sl].T).astype(BF),
            "bq": np.ascontiguousarray(bq[sl]).reshape(1, DH).astype(BF),
            "bk": np.ascontiguousarray(bk[sl]).reshape(1, DH).astype(BF),
            "bv": np.ascontiguousarray(bv[sl]).reshape(1, DH).astype(BF),
            "pat": pat,
        })
    res = run_bass_kernel_spmd(
        nc, in_maps, core_ids=list(range(8)),
        trace=bool(int(os.environ.get("KERNEL_TRACE", "0"))),
    )
    _CACHE["last_result"] = res
    outs = [r["out"] for r in res.results]
    out = np.stack([outs[2 * b] + outs[2 * b + 1] + bo for b in range(4)])
    return out.astype(np.float32)
